# revision 1
# baseline (speedup 1.0000x reference)
"""GQA attention (bs=2, seq=2048, dim=2048, 16 q-heads / 8 kv-heads, hd=128)
on 8 Trainium2 NeuronCores.

Sharding: 2-way data parallel (batch) x 4-way tensor parallel (heads, kv
groups intact).  Core c handles batch c//4 and q-heads [4*(c%4), 4*(c%4)+4)
(kv-heads [2*(c%4), 2*(c%4)+2)).  Each core computes a partial output
projection (row-split wo); the all-reduce over the 4 TP ranks is done on the
host while gathering.

Device kernel (per core, all matmuls in fp32r = full PE rate):
  - host supplies X^T (so `dim` lands on partitions for every projection)
    and rotate-half permuted wq/wk, so RoPE is 3 contiguous-partition DVE
    ops per tile.
  - scores are computed transposed (P^T[k, q]) which makes PV and the
    output projection transpose-free; softmax row-sums come from a
    ones-column matmul, normalization via reciprocal + broadcast matmul.
  - causal masking: k-chunks with k > q_max are skipped entirely; the 4
    diagonal chunk shapes use host-precomputed 0/1 masks multiplied into
    exp(scores).
"""

from contextlib import ExitStack

import ml_dtypes
import numpy as np

import concourse.bass as bass
import concourse.tile as tile
from concourse import bacc, mybir
from concourse.bass_utils import run_bass_kernel_spmd

F32 = mybir.dt.float32
F32R = mybir.dt.float32r
BF16 = mybir.dt.bfloat16

BS = 2
SEQ = 2048
DIM = 2048
N_HEADS = 16
N_KV_HEADS = 8
HD = 128
HALF = HD // 2

NCORES = 8
TP = 4                     # tensor-parallel ranks per batch
NH = N_HEADS // TP         # q heads per core = 4
NKV = N_KV_HEADS // TP     # kv heads per core = 2
QB = 512                   # q block (free dim of score matmuls)
KC = 128                   # k chunk (partition dim of P^T tiles)
DC = 128                   # contraction chunk (partitions)
NDC = DIM // DC            # 16
NB = SEQ // QB             # 4 seq blocks
SCALE = 1.0 / np.sqrt(HD)


def _build_nc():
    nc = bacc.Bacc("TRN2", target_bir_lowering=False, debug=False,
                   num_devices=NCORES)
    xT_d = nc.declare_dram_parameter("xT", [DIM, SEQ], F32R, isOutput=False)
    wq_d = nc.declare_dram_parameter("wq", [DIM, NH * HD], F32R, isOutput=False)
    wk_d = nc.declare_dram_parameter("wk", [DIM, NKV * HD], F32R, isOutput=False)
    wv_d = nc.declare_dram_parameter("wv", [DIM, NKV * HD], F32R, isOutput=False)
    wo_d = nc.declare_dram_parameter("wo", [NH * HD, DIM], BF16, isOutput=False)
    cos_d = nc.declare_dram_parameter("cos2", [HD, SEQ], F32, isOutput=False)
    sin_d = nc.declare_dram_parameter("sins", [HD, SEQ], F32, isOutput=False)
    msk_d = nc.declare_dram_parameter("masks", [KC, KC], BF16, isOutput=False)
    on128_d = nc.declare_dram_parameter("ones128", [128, 1], BF16, isOutput=False)
    on1_d = nc.declare_dram_parameter("ones1", [1, 128], F32R, isOutput=False)
    out_d = nc.declare_dram_parameter("out", [SEQ, DIM], F32, isOutput=True)

    with tile.TileContext(nc) as tc, ExitStack() as ctx:
        wpool = ctx.enter_context(tc.tile_pool(name="weights", bufs=1))
        kvpool = ctx.enter_context(tc.tile_pool(name="kv", bufs=1))
        xpool = ctx.enter_context(tc.tile_pool(name="xt", bufs=16))
        wopool = ctx.enter_context(tc.tile_pool(name="wo", bufs=17))
        qpool = ctx.enter_context(tc.tile_pool(name="qT", bufs=4))
        ppool = ctx.enter_context(tc.tile_pool(name="pT", bufs=6))
        ospool = ctx.enter_context(tc.tile_pool(name="osb", bufs=6))
        cspool = ctx.enter_context(tc.tile_pool(name="cs", bufs=2))
        npool = ctx.enter_context(tc.tile_pool(name="norm", bufs=1))
        tpool = ctx.enter_context(tc.tile_pool(name="tmp", bufs=2))
        obpool = ctx.enter_context(tc.tile_pool(name="outb", bufs=6))
        ps_acc = ctx.enter_context(tc.tile_pool(name="ps_acc", bufs=3,
                                                space="PSUM"))
        ps_sc = ctx.enter_context(tc.tile_pool(name="ps_sc", bufs=3,
                                               space="PSUM"))
        ps_att = ctx.enter_context(tc.tile_pool(name="ps_att", bufs=2,
                                                space="PSUM"))

        # ---- persistent weights/constants in SBUF ----
        wq_sb = wpool.tile([128, NDC * NH * HD], F32R)   # [128, 8192]
        wk_sb = wpool.tile([128, NDC * NKV * HD], F32R)  # [128, 4096]
        wv_sb = wpool.tile([128, NDC * NKV * HD], F32R)  # [128, 4096]

        def issue_block_inputs(j):
            c0 = j * QB
            cos_t = cspool.tile([128, QB], F32, tag="cos", name=f"cos{j}")
            sin_t = cspool.tile([128, QB], F32, tag="sin", name=f"sin{j}")
            nc.gpsimd.dma_start(cos_t[:], cos_d.ap()[:, c0:c0 + QB])
            nc.gpsimd.dma_start(sin_t[:], sin_d.ap()[:, c0:c0 + QB])
            xts = []
            for d in range(NDC):
                xt = xpool.tile([128, QB], F32R, tag="xt", name=f"xt{j}_{d}")
                eng = nc.gpsimd if (j > 0 or d % 2 == 0) else nc.sync
                eng.dma_start(
                    xt[:], xT_d.ap()[d * 128:(d + 1) * 128, c0:c0 + QB])
                xts.append(xt)
            return cos_t, sin_t, xts

        # first Q matmul needs wq chunk 0 + xt(0) — issue those DMAs first
        blk0 = issue_block_inputs(0)
        for d in range(NDC):
            nc.scalar.dma_start(wq_sb[:, d * 512:(d + 1) * 512],
                                wq_d.ap()[d * 128:(d + 1) * 128, :])
        for d in range(NDC):
            nc.scalar.dma_start(wk_sb[:, d * 256:(d + 1) * 256],
                                wk_d.ap()[d * 128:(d + 1) * 128, :])
        for d in range(NDC):
            nc.scalar.dma_start(wv_sb[:, d * 256:(d + 1) * 256],
                                wv_d.ap()[d * 128:(d + 1) * 128, :])
        tri = wpool.tile([KC, KC], BF16, tag="tri", name="tri")
        nc.sync.dma_start(tri[:], msk_d.ap()[:])
        ones128 = wpool.tile([128, 1], BF16, tag="ones128")
        nc.sync.dma_start(ones128[:], on128_d.ap()[:])
        ones1 = wpool.tile([1, 128], F32R, tag="ones1")
        nc.sync.dma_start(ones1[:], on1_d.ap()[:])

        # ---- persistent K^T / V for the whole sequence ----
        kT = [kvpool.tile([128, SEQ], BF16, tag=f"kT{g}", name=f"kT{g}")
              for g in range(NKV)]
        # v_sb columns: [kchunk c][kv head g] -> [:, c*256 + g*128 :+128]
        v_sb = kvpool.tile([128, (SEQ // KC) * NKV * HD], BF16, tag="v")
        assert v_sb.shape[1] == 4096

        def rope(dst, src_ps, cos_t, sin_t):
            """dst = src*cos2 + swap_halves(src)*sins  (dst f32r SBUF)."""
            tmp = tpool.tile([128, QB], F32, tag="ropetmp")
            nc.vector.tensor_mul(tmp[:], src_ps[:], cos_t[:])
            nc.vector.tensor_mul(dst[0:64, :], src_ps[64:128, :],
                                 sin_t[0:64, :])
            nc.vector.tensor_mul(dst[64:128, :], src_ps[0:64, :],
                                 sin_t[64:128, :])
            nc.vector.tensor_add(dst[:], dst[:], tmp[:])

        def q_group(j, h, xts, cos_t, sin_t):
            q_ps = ps_acc.tile([128, QB], F32, tag="acc", name=f"q_ps{j}_{h}")
            for d in range(NDC):
                nc.tensor.matmul(
                    q_ps[:],
                    wq_sb[:, d * 512 + h * 128: d * 512 + (h + 1) * 128],
                    xts[d][:], start=(d == 0), stop=(d == NDC - 1))
            qt = qpool.tile([128, QB], BF16, tag="qT", name=f"qt{j}_{h}")
            rope(qt, q_ps, cos_t, sin_t)
            return qt

        def k_group(j, g, xts, cos_t, sin_t):
            c0 = j * QB
            k_ps = ps_acc.tile([128, QB], F32, tag="acc", name=f"k_ps{j}_{g}")
            for d in range(NDC):
                nc.tensor.matmul(
                    k_ps[:],
                    wk_sb[:, d * 256 + g * 128: d * 256 + (g + 1) * 128],
                    xts[d][:], start=(d == 0), stop=(d == NDC - 1))
            rope(kT[g][:, c0:c0 + QB], k_ps, cos_t, sin_t)

        def v_group(j, m, xts):
            v_ps = ps_acc.tile([128, NKV * HD], F32, tag="acc",
                               name=f"v_ps{j}_{m}")
            for d in range(NDC):
                nc.tensor.matmul(
                    v_ps[:],
                    xts[d][:, m * 128:(m + 1) * 128],
                    wv_sb[:, d * 256:(d + 1) * 256],
                    start=(d == 0), stop=(d == NDC - 1))
            kc = 4 * j + m
            nc.scalar.copy(v_sb[:, kc * 256:(kc + 1) * 256], v_ps[:])

        def prefetch_wo(j):
            wo_ts = []
            for n in range(4):
                row = []
                for h in range(NH):
                    wo_t = wopool.tile([128, 512], BF16, tag="wo",
                                       name=f"wo{j}_{n}_{h}")
                    nc.gpsimd.dma_start(
                        wo_t[:], wo_d.ap()[h * 128:(h + 1) * 128,
                                           n * 512:(n + 1) * 512])
                    row.append(wo_t)
                wo_ts.append(row)
            return wo_ts

        def attn_head(j, nkc, qT, h):
            g = h // 2
            o_ps = ps_att.tile([128, QB], F32, tag="att", name=f"o_ps{j}_{h}")
            z_ps = ps_att.tile([1, QB], F32, tag="att", name=f"z_ps{j}_{h}")
            for kc in range(nkc):
                # on diagonal chunks only q >= kc*128 is unmasked: shrink
                # every op to the live q-subrange [off, 512)
                off = max(0, (kc - 4 * j) * 128)
                sc_ps = ps_sc.tile([128, QB], F32, tag="sc",
                                   name=f"sc{j}_{h}_{kc}")
                nc.tensor.matmul(sc_ps[:, off:QB],
                                 kT[g][:, kc * 128:(kc + 1) * 128],
                                 qT[h][:, off:QB], start=True, stop=True)
                pt = ppool.tile([128, QB], BF16, tag="pT",
                                name=f"pt{j}_{h}_{kc}")
                nc.scalar.activation(pt[:, off:QB], sc_ps[:, off:QB],
                                     mybir.ActivationFunctionType.Exp,
                                     scale=float(SCALE))
                if kc >= 4 * j:
                    nc.vector.tensor_mul(pt[:, off:off + KC],
                                         pt[:, off:off + KC], tri[:])
                nc.tensor.matmul(o_ps[:, off:QB],
                                 v_sb[:, kc * 256 + g * 128:
                                      kc * 256 + (g + 1) * 128],
                                 pt[:, off:QB], start=(kc == 0),
                                 stop=(kc == nkc - 1))
                nc.tensor.matmul(z_ps[:, off:QB], ones128[:], pt[:, off:QB],
                                 start=(kc == 0), stop=(kc == nkc - 1))
            # stage unnormalized O' and 1/z; the normalization matmul is
            # deferred so no PE instruction here waits on this chain
            o_sb = ospool.tile([128, QB], BF16, tag="osb", name=f"o_sb{j}_{h}")
            nc.scalar.copy(o_sb[:], o_ps[:])
            z_sb = npool.tile([1, QB], F32, tag="z", bufs=4,
                              name=f"z_sb{j}_{h}")
            nc.scalar.copy(z_sb[:], z_ps[:])
            # reshape the z row to [128,4] so the reciprocal runs on all 128
            # DVE lanes (~100ns) instead of 3.4us on one lane (which would
            # block the mask multiplies behind it in the DVE FIFO)
            zc = npool.tile([128, QB // 128], F32, tag="zc", bufs=4,
                            name=f"zc{j}_{h}")
            nc.gpsimd.dma_start(zc[:], z_sb[:])
            rzc = npool.tile([128, QB // 128], F32R, tag="rzc", bufs=4,
                             name=f"rzc{j}_{h}")
            with nc.allow_low_precision(
                    reason="1/z in fp32r (11-bit mantissa) is plenty"):
                nc.vector.reciprocal(rzc[:], zc[:])
            rz = npool.tile([1, QB], F32R, tag="rz", bufs=6,
                            name=f"rz{j}_{h}")
            nc.gpsimd.dma_start(rz[:], rzc[:])
            return (o_sb, rz)

        def norm_head(j, h, ot):
            # o_sb *= broadcast(1/z) (in place)
            o_sb, rz = ot
            zb_ps = ps_sc.tile([128, QB], F32, tag="sc", name=f"zb{j}_{h}")
            nc.tensor.matmul(zb_ps[:], ones1[:], rz[:], start=True, stop=True)
            nc.vector.tensor_mul(o_sb[:], o_sb[:], zb_ps[:])

        def outproj_block(j, oT, wo_ts, skip_norm=()):
            c0 = j * QB
            for h in range(NH):
                if h not in skip_norm:
                    norm_head(j, h, oT[h])
            for n in range(4):
                for mp in range(2):
                    op_ps = [ps_acc.tile([128, 512], F32, tag="acc",
                                         name=f"op{j}_{n}_{mp}_{m}")
                             for m in range(2)]
                    for h in range(NH):
                        for mi in range(2):
                            m = 2 * mp + mi
                            nc.tensor.matmul(
                                op_ps[mi][:],
                                oT[h][0][:, m * 128:(m + 1) * 128],
                                wo_ts[n][h][:],
                                start=(h == 0), stop=(h == NH - 1))
                    for mi in range(2):
                        m = 2 * mp + mi
                        ob = obpool.tile([128, 512], F32, tag="ob",
                                         name=f"ob{j}_{n}_{m}")
                        nc.vector.tensor_copy(ob[:], op_ps[mi][:])
                        nc.sync.dma_start(
                            out_d.ap()[c0 + m * 128: c0 + (m + 1) * 128,
                                       n * 512:(n + 1) * 512], ob[:])

        # ---- software pipeline ----
        # block 0 QKV up front; then for each j: attention(j) heads
        # interleaved with block-(j+1) Q projection groups (dense PE work
        # hides the exp/mask chains), then K/V of j+1, then outproj(j).
        cos_t, sin_t, xts = blk0
        qT_cur = [q_group(0, h, xts, cos_t, sin_t) for h in range(NH)]
        for g in range(NKV):
            k_group(0, g, xts, cos_t, sin_t)
        for m in range(4):
            v_group(0, m, xts)

        oT3a = None
        for j in range(NB - 1):
            nkc = 4 * (j + 1)
            wo_cur = prefetch_wo(j)
            cosn, sinn, xtsn = issue_block_inputs(j + 1)
            oT_cur = [attn_head(j, nkc, qT_cur, 0),
                      attn_head(j, nkc, qT_cur, 1)]
            qT_next = [q_group(j + 1, 0, xtsn, cosn, sinn)]
            oT_cur.append(attn_head(j, nkc, qT_cur, 2))
            qT_next.append(q_group(j + 1, 1, xtsn, cosn, sinn))
            oT_cur.append(attn_head(j, nkc, qT_cur, 3))
            qT_next.append(q_group(j + 1, 2, xtsn, cosn, sinn))
            qT_next.append(q_group(j + 1, 3, xtsn, cosn, sinn))
            for g in range(NKV):
                k_group(j + 1, g, xtsn, cosn, sinn)
            for m in range(4):
                v_group(j + 1, m, xtsn)
            qT_cur = qT_next
            if j == NB - 2:
                # run half of the last block's attention before outproj(j)
                # so its exp-bound tail overlaps matmul work
                wo3 = prefetch_wo(NB - 1)
                oT3a = [attn_head(NB - 1, 4 * NB, qT_cur, 0),
                        attn_head(NB - 1, 4 * NB, qT_cur, 1)]
            outproj_block(j, oT_cur, wo_cur)
        for h in range(2):
            norm_head(NB - 1, h, oT3a[h])
        oT3b = [attn_head(NB - 1, 4 * NB, qT_cur, 2),
                attn_head(NB - 1, 4 * NB, qT_cur, 3)]
        outproj_block(NB - 1, oT3a + oT3b, wo3, skip_norm=(0, 1))

    nc.compile()
    return nc


_NC_CACHE = None


def _get_nc():
    global _NC_CACHE
    if _NC_CACHE is None:
        _NC_CACHE = _build_nc()
    return _NC_CACHE


def _host_prep(inputs):
    """Build the 8 per-core input maps from the full problem inputs."""
    hs = np.asarray(inputs["hidden_state"], dtype=np.float32)
    cos = np.asarray(inputs["freq_cos"], dtype=np.float32)[0, :, 0, :]  # [S,64]
    sin = np.asarray(inputs["freq_sin"], dtype=np.float32)[0, :, 0, :]
    wq = np.asarray(inputs["wq"], dtype=np.float32)
    wk = np.asarray(inputs["wk"], dtype=np.float32)
    wv = np.asarray(inputs["wv"], dtype=np.float32)
    wo = np.asarray(inputs["wo"], dtype=np.float32)

    perm = np.concatenate([np.arange(0, HD, 2), np.arange(1, HD, 2)])  # [128]

    cos2 = np.empty((HD, SEQ), dtype=np.float32)
    sins = np.empty((HD, SEQ), dtype=np.float32)
    cos2[:HALF] = cos.T
    cos2[HALF:] = cos.T
    sins[:HALF] = -sin.T
    sins[HALF:] = sin.T

    ki = np.arange(KC)
    masks = (ki[:, None] <= ki[None, :]).astype(ml_dtypes.bfloat16)

    ones128 = np.ones((128, 1), dtype=ml_dtypes.bfloat16)
    ones1 = np.ones((1, 128), dtype=np.float32)

    xTs = [np.ascontiguousarray(hs[b].T) for b in range(BS)]

    in_maps = []
    for c in range(NCORES):
        b, r = divmod(c, TP)
        qcols = np.concatenate(
            [(4 * r + h) * HD + perm for h in range(NH)])
        kcols = np.concatenate(
            [(NKV * r + g) * HD + perm for g in range(NKV)])
        vcols = np.concatenate(
            [(NKV * r + g) * HD + np.arange(HD) for g in range(NKV)])
        worows = np.concatenate(
            [(4 * r + h) * HD + np.arange(HD) for h in range(NH)])
        in_maps.append({
            "xT": xTs[b],
            "wq": np.ascontiguousarray(wq[:, qcols]),
            "wk": np.ascontiguousarray(wk[:, kcols]),
            "wv": np.ascontiguousarray(wv[:, vcols]),
            "wo": np.ascontiguousarray(wo[worows, :]).astype(ml_dtypes.bfloat16),
            "cos2": cos2,
            "sins": sins,
            "masks": masks,
            "ones128": ones128,
            "ones1": ones1,
        })
    return in_maps


def _run(inputs, trace=False, **trace_kwargs):
    nc = _get_nc()
    in_maps = _host_prep(inputs)
    res = run_bass_kernel_spmd(nc, in_maps, list(range(NCORES)),
                               trace=trace, **trace_kwargs)
    out = np.zeros((BS, SEQ, DIM), dtype=np.float32)
    for c in range(NCORES):
        out[c // TP] += res.results[c]["out"]
    return out, res


def kernel(**inputs) -> np.ndarray:
    out, _ = _run(inputs, trace=False)
    return out



# revision 3
# speedup vs baseline: 1.1180x; 1.1180x over previous
"""GQA attention (bs=2, seq=2048, dim=2048, 16 q-heads / 8 kv-heads, hd=128)
on 8 Trainium2 NeuronCores.

Sharding: 2-way data parallel (batch) x 4-way tensor parallel (heads, kv
groups intact).  Core c handles batch c//4 and q-heads [4*(c%4), 4*(c%4)+4)
(kv-heads [2*(c%4), 2*(c%4)+2)).  Each core computes a partial output
projection (row-split wo); the all-reduce over the 4 TP ranks is done on the
host while gathering (bf16 partials summed in f32).

Device kernel (per core):
  - all inputs bf16 (weights, x^T) -> FWL-eligible stationaries, half DMA.
  - host supplies X^T (so `dim` lands on partitions for every projection)
    and rotate-half permuted wq/wk, so RoPE is 3 contiguous-partition DVE
    ops per tile.
  - scores are computed transposed (P^T[k, q]) which makes PV and the
    output projection transpose-free.
  - causal masking is additive: a [128,128] -1e9 strictly-lower matrix is
    accumulated into the scores PSUM bank by a tiny N=128 matmul
    (identity stationary) before the score matmul, so exp() produces
    exact zeros and the DVE mask multiply disappears from the
    exp->PV chain.
  - softmax row-sums: P^T chunks are accumulated into a [128, QB] f32r
    SBUF tile by DVE adds; one all-ones [128,128] matmul per head-block
    then reduces over partitions AND broadcasts z to all 128 partitions
    in a single N=512 matmul; normalization is fused into the PSUM->SBUF
    copy of the attention output (o_sb = o_ps * reciprocal(z)).
"""

from contextlib import ExitStack

import ml_dtypes
import numpy as np

import concourse.bass as bass
import concourse.tile as tile
from concourse import bacc, mybir
from concourse.bass_utils import run_bass_kernel_spmd

F32 = mybir.dt.float32
F32R = mybir.dt.float32r
BF16 = mybir.dt.bfloat16

BS = 2
SEQ = 2048
DIM = 2048
N_HEADS = 16
N_KV_HEADS = 8
HD = 128
HALF = HD // 2

NCORES = 8
TP = 4                     # tensor-parallel ranks per batch
NH = N_HEADS // TP         # q heads per core = 4
NKV = N_KV_HEADS // TP     # kv heads per core = 2
QB = 512                   # q block (free dim of score matmuls)
KC = 128                   # k chunk (partition dim of P^T tiles)
DC = 128                   # contraction chunk (partitions)
NDC = DIM // DC            # 16
NB = SEQ // QB             # 4 seq blocks
SCALE = 1.0 / np.sqrt(HD)


def _build_nc():
    nc = bacc.Bacc("TRN2", target_bir_lowering=False, debug=False,
                   num_devices=NCORES)
    xT_d = nc.declare_dram_parameter("xT", [DIM, SEQ], BF16, isOutput=False)
    wq_d = nc.declare_dram_parameter("wq", [DIM, NH * HD], BF16, isOutput=False)
    wk_d = nc.declare_dram_parameter("wk", [DIM, NKV * HD], BF16, isOutput=False)
    wv_d = nc.declare_dram_parameter("wv", [DIM, NKV * HD], BF16, isOutput=False)
    wo_d = nc.declare_dram_parameter("wo", [NH * HD, DIM], BF16, isOutput=False)
    cos_d = nc.declare_dram_parameter("cos2", [HD, SEQ], F32, isOutput=False)
    sin_d = nc.declare_dram_parameter("sins", [HD, SEQ], F32, isOutput=False)
    msk_d = nc.declare_dram_parameter("maskadd", [KC, KC], BF16, isOutput=False)
    idn_d = nc.declare_dram_parameter("ident", [KC, KC], BF16, isOutput=False)
    ones_d = nc.declare_dram_parameter("onessq", [128, 128], F32R, isOutput=False)
    out_d = nc.declare_dram_parameter("out", [SEQ, DIM], BF16, isOutput=True)

    with tile.TileContext(nc) as tc, ExitStack() as ctx:
        wpool = ctx.enter_context(tc.tile_pool(name="weights", bufs=1))
        kvpool = ctx.enter_context(tc.tile_pool(name="kv", bufs=1))
        xpool = ctx.enter_context(tc.tile_pool(name="xt", bufs=24))
        wopool = ctx.enter_context(tc.tile_pool(name="wo", bufs=33))
        qpool = ctx.enter_context(tc.tile_pool(name="qT", bufs=8))
        ppool = ctx.enter_context(tc.tile_pool(name="pT", bufs=8))
        ospool = ctx.enter_context(tc.tile_pool(name="osb", bufs=8))
        cspool = ctx.enter_context(tc.tile_pool(name="cs", bufs=2))
        zpool = ctx.enter_context(tc.tile_pool(name="zacc", bufs=3))
        rzpool = ctx.enter_context(tc.tile_pool(name="rz", bufs=3))
        tpool = ctx.enter_context(tc.tile_pool(name="tmp", bufs=2))
        obpool = ctx.enter_context(tc.tile_pool(name="outb", bufs=8))
        ps_acc = ctx.enter_context(tc.tile_pool(name="ps_acc", bufs=3,
                                                space="PSUM"))
        ps_sc = ctx.enter_context(tc.tile_pool(name="ps_sc", bufs=3,
                                               space="PSUM"))
        ps_att = ctx.enter_context(tc.tile_pool(name="ps_att", bufs=2,
                                                space="PSUM"))

        # ---- persistent weights/constants in SBUF (one tile per d-chunk
        # so the first matmuls only wait on their own chunk's DMA) ----
        wq_t = [wpool.tile([128, NH * HD], BF16, tag=f"wq{d}", name=f"wq{d}")
                for d in range(NDC)]
        wk_t = [wpool.tile([128, NKV * HD], BF16, tag=f"wk{d}", name=f"wk{d}")
                for d in range(NDC)]
        wv_t = [wpool.tile([128, NKV * HD], BF16, tag=f"wv{d}", name=f"wv{d}")
                for d in range(NDC)]

        def issue_block_inputs(j):
            c0 = j * QB
            cos_t = cspool.tile([128, QB], F32, tag="cos", name=f"cos{j}")
            sin_t = cspool.tile([128, QB], F32, tag="sin", name=f"sin{j}")
            nc.gpsimd.dma_start(cos_t[:], cos_d.ap()[:, c0:c0 + QB])
            nc.gpsimd.dma_start(sin_t[:], sin_d.ap()[:, c0:c0 + QB])
            xts = []
            for d in range(NDC):
                xt = xpool.tile([128, QB], BF16, tag="xt", name=f"xt{j}_{d}")
                eng = nc.gpsimd if (j > 0 or d % 2 == 0) else nc.sync
                eng.dma_start(
                    xt[:], xT_d.ap()[d * 128:(d + 1) * 128, c0:c0 + QB])
                xts.append(xt)
            return cos_t, sin_t, xts

        # first Q matmul needs wq chunk 0 + xt(0)_0 — issue those DMAs first
        nc.scalar.dma_start(wq_t[0][:], wq_d.ap()[0:128, :])
        blk0 = issue_block_inputs(0)
        for d in range(1, NDC):
            nc.scalar.dma_start(wq_t[d][:],
                                wq_d.ap()[d * 128:(d + 1) * 128, :])
        ident = wpool.tile([KC, KC], BF16, tag="ident")
        nc.sync.dma_start(ident[:], idn_d.ap()[:])
        maskA = wpool.tile([KC, KC], BF16, tag="maskA")
        nc.sync.dma_start(maskA[:], msk_d.ap()[:])
        ones_sq = wpool.tile([128, 128], F32R, tag="onessq")
        nc.sync.dma_start(ones_sq[:], ones_d.ap()[:])
        for d in range(NDC):
            nc.sync.dma_start(wk_t[d][:], wk_d.ap()[d * 128:(d + 1) * 128, :])
        for d in range(NDC):
            nc.sync.dma_start(wv_t[d][:], wv_d.ap()[d * 128:(d + 1) * 128, :])

        # ---- persistent K^T / V for the whole sequence ----
        kT = [kvpool.tile([128, SEQ], BF16, tag=f"kT{g}", name=f"kT{g}")
              for g in range(NKV)]
        # v_sb columns: [kchunk c][kv head g] -> [:, c*256 + g*128 :+128]
        v_sb = kvpool.tile([128, (SEQ // KC) * NKV * HD], BF16, tag="v")
        assert v_sb.shape[1] == 4096

        def rope(dst, src_ps, cos_t, sin_t):
            """dst = src*cos2 + swap_halves(src)*sins  (dst bf16 SBUF)."""
            tmp = tpool.tile([128, QB], F32, tag="ropetmp")
            nc.vector.tensor_mul(tmp[:], src_ps[:], cos_t[:])
            nc.vector.tensor_mul(dst[0:64, :], src_ps[64:128, :],
                                 sin_t[0:64, :])
            nc.vector.tensor_mul(dst[64:128, :], src_ps[0:64, :],
                                 sin_t[64:128, :])
            nc.vector.tensor_add(dst[:], dst[:], tmp[:])

        def q_group(j, h, xts, cos_t, sin_t):
            q_ps = ps_acc.tile([128, QB], F32, tag="acc", name=f"q_ps{j}_{h}")
            for d in range(NDC):
                nc.tensor.matmul(
                    q_ps[:],
                    wq_t[d][:, h * 128:(h + 1) * 128],
                    xts[d][:], start=(d == 0), stop=(d == NDC - 1))
            qt = qpool.tile([128, QB], BF16, tag="qT", name=f"qt{j}_{h}")
            rope(qt, q_ps, cos_t, sin_t)
            return qt

        def k_group(j, g, xts, cos_t, sin_t):
            c0 = j * QB
            k_ps = ps_acc.tile([128, QB], F32, tag="acc", name=f"k_ps{j}_{g}")
            for d in range(NDC):
                nc.tensor.matmul(
                    k_ps[:],
                    wk_t[d][:, g * 128:(g + 1) * 128],
                    xts[d][:], start=(d == 0), stop=(d == NDC - 1))
            rope(kT[g][:, c0:c0 + QB], k_ps, cos_t, sin_t)

        def v_group(j, m, xts):
            v_ps = ps_acc.tile([128, NKV * HD], F32, tag="acc",
                               name=f"v_ps{j}_{m}")
            for d in range(NDC):
                nc.tensor.matmul(
                    v_ps[:],
                    xts[d][:, m * 128:(m + 1) * 128],
                    wv_t[d][:],
                    start=(d == 0), stop=(d == NDC - 1))
            kc = 4 * j + m
            nc.scalar.copy(v_sb[:, kc * 256:(kc + 1) * 256], v_ps[:])

        def prefetch_wo(j):
            wo_ts = []
            for n in range(4):
                row = []
                for h in range(NH):
                    wo_t = wopool.tile([128, 512], BF16, tag="wo",
                                       name=f"wo{j}_{n}_{h}")
                    nc.gpsimd.dma_start(
                        wo_t[:], wo_d.ap()[h * 128:(h + 1) * 128,
                                           n * 512:(n + 1) * 512])
                    row.append(wo_t)
                wo_ts.append(row)
            return wo_ts

        def attn_head(j, nkc, qT, h):
            g = h // 2
            o_ps = ps_att.tile([128, QB], F32, tag="att", name=f"o_ps{j}_{h}")
            zacc = zpool.tile([128, QB], F32R, tag="zacc",
                              name=f"zacc{j}_{h}")
            for kc in range(nkc):
                # on diagonal chunks only q >= kc*128 is unmasked: shrink
                # every op to the live q-subrange [off, 512)
                off = max(0, (kc - 4 * j) * 128)
                sc_ps = ps_sc.tile([128, QB], F32, tag="sc",
                                   name=f"sc{j}_{h}_{kc}")
                if kc >= 4 * j:
                    # additive causal mask: -1e9 above the diagonal, via a
                    # tiny identity-stationary matmul into the same bank
                    nc.tensor.matmul(sc_ps[:, off:off + KC], ident[:],
                                     maskA[:], start=True, stop=False)
                    nc.tensor.matmul(sc_ps[:, off:QB],
                                     kT[g][:, kc * 128:(kc + 1) * 128],
                                     qT[h][:, off:QB], start=False, stop=True)
                else:
                    nc.tensor.matmul(sc_ps[:, off:QB],
                                     kT[g][:, kc * 128:(kc + 1) * 128],
                                     qT[h][:, off:QB], start=True, stop=True)
                pt = ppool.tile([128, QB], BF16, tag="pT",
                                name=f"pt{j}_{h}_{kc}")
                nc.scalar.activation(pt[:, off:QB], sc_ps[:, off:QB],
                                     mybir.ActivationFunctionType.Exp,
                                     scale=float(SCALE))
                with nc.allow_low_precision(
                        reason="softmax z accum in f32r is plenty"):
                    if kc == 0:
                        nc.vector.tensor_copy(zacc[:], pt[:])
                    else:
                        nc.vector.tensor_add(zacc[:, off:QB],
                                             zacc[:, off:QB], pt[:, off:QB])
                nc.tensor.matmul(o_ps[:, off:QB],
                                 v_sb[:, kc * 256 + g * 128:
                                      kc * 256 + (g + 1) * 128],
                                 pt[:, off:QB], start=(kc == 0),
                                 stop=(kc == nkc - 1))
            # one matmul reduces z over partitions AND broadcasts it to all
            # 128 partitions (all-ones stationary)
            zb_ps = ps_sc.tile([128, QB], F32, tag="sc", name=f"zb{j}_{h}")
            nc.tensor.matmul(zb_ps[:], ones_sq[:], zacc[:],
                             start=True, stop=True)
            rz = rzpool.tile([128, QB], F32R, tag="rz", name=f"rz{j}_{h}")
            with nc.allow_low_precision(
                    reason="1/z in fp32r (11-bit mantissa) is plenty"):
                nc.vector.reciprocal(rz[:], zb_ps[:])
            # normalization fused into the PSUM->SBUF copy
            o_sb = ospool.tile([128, QB], BF16, tag="osb", name=f"o_sb{j}_{h}")
            nc.vector.tensor_mul(o_sb[:], o_ps[:], rz[:])
            return o_sb

        def outproj_block(j, oT, wo_ts):
            c0 = j * QB
            for n in range(4):
                for mp in range(2):
                    op_ps = [ps_acc.tile([128, 512], F32, tag="acc",
                                         name=f"op{j}_{n}_{mp}_{m}")
                             for m in range(2)]
                    for h in range(NH):
                        for mi in range(2):
                            m = 2 * mp + mi
                            nc.tensor.matmul(
                                op_ps[mi][:],
                                oT[h][:, m * 128:(m + 1) * 128],
                                wo_ts[n][h][:],
                                start=(h == 0), stop=(h == NH - 1))
                    for mi in range(2):
                        m = 2 * mp + mi
                        ob = obpool.tile([128, 512], BF16, tag="ob",
                                         name=f"ob{j}_{n}_{m}")
                        nc.vector.tensor_copy(ob[:], op_ps[mi][:])
                        nc.sync.dma_start(
                            out_d.ap()[c0 + m * 128: c0 + (m + 1) * 128,
                                       n * 512:(n + 1) * 512], ob[:])

        # ---- software pipeline ----
        # block 0 QKV up front; then for each j: attention(j) heads
        # interleaved with block-(j+1) Q projection groups (dense PE work
        # hides the exp chains), then K/V of j+1, then outproj(j).
        cos_t, sin_t, xts = blk0
        qT_cur = [q_group(0, h, xts, cos_t, sin_t) for h in range(NH)]
        for g in range(NKV):
            k_group(0, g, xts, cos_t, sin_t)
        for m in range(4):
            v_group(0, m, xts)

        oT3 = None
        for j in range(NB - 1):
            nkc = 4 * (j + 1)
            wo_cur = prefetch_wo(j)
            cosn, sinn, xtsn = issue_block_inputs(j + 1)
            oT_cur = [attn_head(j, nkc, qT_cur, 0),
                      attn_head(j, nkc, qT_cur, 1)]
            qT_next = [q_group(j + 1, 0, xtsn, cosn, sinn)]
            oT_cur.append(attn_head(j, nkc, qT_cur, 2))
            qT_next.append(q_group(j + 1, 1, xtsn, cosn, sinn))
            oT_cur.append(attn_head(j, nkc, qT_cur, 3))
            qT_next.append(q_group(j + 1, 2, xtsn, cosn, sinn))
            qT_next.append(q_group(j + 1, 3, xtsn, cosn, sinn))
            for g in range(NKV):
                k_group(j + 1, g, xtsn, cosn, sinn)
            for m in range(4):
                v_group(j + 1, m, xtsn)
            qT_cur = qT_next
            if j == NB - 2:
                # run 3 of the last block's 4 heads before outproj(j) so
                # their exp-bound chains overlap dense outproj matmul work
                wo3 = prefetch_wo(NB - 1)
                oT3 = [attn_head(NB - 1, 4 * NB, qT_cur, 0),
                       attn_head(NB - 1, 4 * NB, qT_cur, 1),
                       attn_head(NB - 1, 4 * NB, qT_cur, 2)]
            outproj_block(j, oT_cur, wo_cur)
        oT3.append(attn_head(NB - 1, 4 * NB, qT_cur, 3))
        outproj_block(NB - 1, oT3, wo3)

    nc.compile()
    return nc


_NC_CACHE = None


def _get_nc():
    global _NC_CACHE
    if _NC_CACHE is None:
        _NC_CACHE = _build_nc()
    return _NC_CACHE


def _host_prep(inputs):
    """Build the 8 per-core input maps from the full problem inputs."""
    hs = np.asarray(inputs["hidden_state"], dtype=np.float32)
    cos = np.asarray(inputs["freq_cos"], dtype=np.float32)[0, :, 0, :]  # [S,64]
    sin = np.asarray(inputs["freq_sin"], dtype=np.float32)[0, :, 0, :]
    wq = np.asarray(inputs["wq"], dtype=np.float32)
    wk = np.asarray(inputs["wk"], dtype=np.float32)
    wv = np.asarray(inputs["wv"], dtype=np.float32)
    wo = np.asarray(inputs["wo"], dtype=np.float32)

    perm = np.concatenate([np.arange(0, HD, 2), np.arange(1, HD, 2)])  # [128]

    cos2 = np.empty((HD, SEQ), dtype=np.float32)
    sins = np.empty((HD, SEQ), dtype=np.float32)
    cos2[:HALF] = cos.T
    cos2[HALF:] = cos.T
    sins[:HALF] = -sin.T
    sins[HALF:] = sin.T

    ki = np.arange(KC)
    # additive causal mask for diagonal chunks: key k > query c -> -1e9
    maskadd = np.where(ki[:, None] > ki[None, :], -1e9,
                       0.0).astype(ml_dtypes.bfloat16)
    ident = np.eye(KC, dtype=ml_dtypes.bfloat16)
    onessq = np.ones((128, 128), dtype=np.float32)

    xTs = [np.ascontiguousarray(hs[b].T).astype(ml_dtypes.bfloat16)
           for b in range(BS)]

    in_maps = []
    for c in range(NCORES):
        b, r = divmod(c, TP)
        qcols = np.concatenate(
            [(4 * r + h) * HD + perm for h in range(NH)])
        kcols = np.concatenate(
            [(NKV * r + g) * HD + perm for g in range(NKV)])
        vcols = np.concatenate(
            [(NKV * r + g) * HD + np.arange(HD) for g in range(NKV)])
        worows = np.concatenate(
            [(4 * r + h) * HD + np.arange(HD) for h in range(NH)])
        in_maps.append({
            "xT": xTs[b],
            "wq": np.ascontiguousarray(wq[:, qcols]).astype(ml_dtypes.bfloat16),
            "wk": np.ascontiguousarray(wk[:, kcols]).astype(ml_dtypes.bfloat16),
            "wv": np.ascontiguousarray(wv[:, vcols]).astype(ml_dtypes.bfloat16),
            "wo": np.ascontiguousarray(wo[worows, :]).astype(ml_dtypes.bfloat16),
            "cos2": cos2,
            "sins": sins,
            "maskadd": maskadd,
            "ident": ident,
            "onessq": onessq,
        })
    return in_maps


def _run(inputs, trace=False, **trace_kwargs):
    nc = _get_nc()
    in_maps = _host_prep(inputs)
    res = run_bass_kernel_spmd(nc, in_maps, list(range(NCORES)),
                               trace=trace, **trace_kwargs)
    out = np.zeros((BS, SEQ, DIM), dtype=np.float32)
    for c in range(NCORES):
        out[c // TP] += np.asarray(res.results[c]["out"], dtype=np.float32)
    return out, res


def kernel(**inputs) -> np.ndarray:
    out, _ = _run(inputs, trace=False)
    return out
